# revision 3
# baseline (speedup 1.0000x reference)
"""ConvLSTM cell forward on 8 Trainium2 NeuronCores.

Problem: B=16, Cin=64, Chid=128, H=W=64, K=3 (SAME padding).
  ig = sigmoid(conv(x,Wxi) + bxi + conv(h,Whi) + Wci*c)
  fg = sigmoid(conv(x,Wxf) + bxf + conv(h,Whf) + Wcf*c)
  c_new = fg*c + ig*tanh(conv(x,Wxc) + bxc + conv(h,Whc))
  og = sigmoid(conv(x,Wxo) + bxo + conv(h,Who) + Wco*c)
  h_new = og*c_new
  returns (og, h_new, c_new)

Strategy:
  - Data-parallel over batch: 2 images per core, weights replicated.
  - Conv as matmul over channel dim: inputs stored channel-on-partition with
    a zero-padded (H+2)x(W+2) spatial layout flattened with row stride 66.
    A 3x3 tap (dy,dx) is then a constant flat offset, so each tap is one
    contiguous matmul rhs slice; 18 matmuls (9 taps x {x,h}) accumulate one
    gate's pre-activation in PSUM.
  - Output computed in the same padded-stride layout, 4 rows (N=264) per
    PSUM bank; the 2 garbage columns per row are skipped by strided views.
  - Matmuls run in float32r (fp32 with 11-bit mantissa, ~4x fp32 speed);
    inputs/weights are pre-rounded to fp32r on the host. Elementwise math
    and accumulation stay fp32.
"""

import os
import numpy as np

B, CIN, CHID, H, W, K = 16, 64, 128, 64, 64, 3
N_CORES = 8
PER = B // N_CORES          # images per core
WPAD = W + 2                # padded row stride
FLAT = (H + 2) * WPAD + 4   # padded flat length (+4 tail pad for tap overread)
ROWS = 4                    # output rows per PSUM chunk
NCHUNK = H // ROWS
CHUNK_N = ROWS * WPAD       # 264 (even, >=256 for full-rate fp32r)
CC = ROWS * W               # 256 valid elements per chunk
HW = H * W

_PROG = None
LAST_RESULTS = None


def _round_fp32r(a):
    """Round fp32 array to fp32r (11 mantissa bits, round-half-up)."""
    b = np.ascontiguousarray(a, dtype=np.float32).view(np.uint32).astype(np.uint64)
    r = ((b + 0x800) & ~np.uint64(0xFFF)).astype(np.uint32)
    return r.view(np.float32)


def _pad_flat(a):
    """[N, C, H, W] fp32 -> [N, C, FLAT] zero-padded 66-stride layout."""
    n, c = a.shape[0], a.shape[1]
    out = np.zeros((n, c, FLAT), dtype=np.float32)
    p = out[:, :, : (H + 2) * WPAD].reshape(n, c, H + 2, WPAD)
    p[:, :, 1 : H + 1, 1 : W + 1] = a
    return out


def _build_program():
    import concourse.bacc as bacc
    import concourse.tile as tile
    import concourse.mybir as mybir
    from contextlib import ExitStack

    f32 = mybir.dt.float32
    f32r = mybir.dt.float32r

    nc = bacc.Bacc("TRN2", target_bir_lowering=False, debug=False,
                   num_devices=N_CORES)

    xp_d = nc.dram_tensor("xp", [PER, CIN, FLAT], f32r, kind="ExternalInput").ap()
    hp_d = nc.dram_tensor("hp", [PER, CHID, FLAT], f32r, kind="ExternalInput").ap()
    c_d = nc.dram_tensor("c", [PER, CHID, HW], f32, kind="ExternalInput").ap()
    wx_d = nc.dram_tensor("wx", [4, CIN, 9 * CHID], f32r, kind="ExternalInput").ap()
    wh_d = nc.dram_tensor("wh", [4, CHID, 9 * CHID], f32r, kind="ExternalInput").ap()
    bias_d = nc.dram_tensor("bias", [CHID, 4], f32, kind="ExternalInput").ap()
    peep_d = nc.dram_tensor("peep", [3, CHID, HW], f32, kind="ExternalInput").ap()
    og_d = nc.dram_tensor("og", [PER, CHID, HW], f32, kind="ExternalOutput").ap()
    hn_d = nc.dram_tensor("hn", [PER, CHID, HW], f32, kind="ExternalOutput").ap()
    cn_d = nc.dram_tensor("cn", [PER, CHID, HW], f32, kind="ExternalOutput").ap()

    SIG = mybir.ActivationFunctionType.Sigmoid
    TANH = mybir.ActivationFunctionType.Tanh

    with tile.TileContext(nc) as tc, ExitStack() as ctx:
        const = ctx.enter_context(tc.tile_pool(name="const", bufs=1))
        imgs = ctx.enter_context(tc.tile_pool(name="imgs", bufs=2))
        work = ctx.enter_context(tc.tile_pool(name="work", bufs=2))
        outs = ctx.enter_context(tc.tile_pool(name="outs", bufs=2))
        psum = ctx.enter_context(tc.tile_pool(name="psum", bufs=8, space="PSUM"))

        wx_t = [const.tile([CIN, 9 * CHID], f32r, tag=f"wx{g}", name=f"wx{g}") for g in range(4)]
        wh_t = [const.tile([CHID, 9 * CHID], f32r, tag=f"wh{g}", name=f"wh{g}") for g in range(4)]
        for g in range(4):
            nc.sync.dma_start(wx_t[g][:], wx_d[g])
            nc.sync.dma_start(wh_t[g][:], wh_d[g])
        bias_t = const.tile([CHID, 4], f32)
        nc.sync.dma_start(bias_t[:], bias_d)
        peep_t = [const.tile([CHID, HW], f32, tag=f"peep{j}", name=f"peep{j}") for j in range(3)]
        for j in range(3):
            nc.sync.dma_start(peep_t[j][:], peep_d[j])

        for b in range(PER):
            xp = imgs.tile([CIN, FLAT], f32r, tag="xp")
            nc.sync.dma_start(xp[:], xp_d[b])
            hp = imgs.tile([CHID, FLAT], f32r, tag="hp")
            nc.sync.dma_start(hp[:], hp_d[b])
            for kc in range(NCHUNK):
                o0 = kc * CHUNK_N
                # gate order: 0=i, 1=f, 2=o, 3=candidate
                ps = [psum.tile([CHID, CHUNK_N], f32, tag="ps", name=f"ps{b}_{kc}_{_g}") for _g in range(4)]
                for g in range(4):
                    for tap in range(9):
                        dy, dx = divmod(tap, 3)
                        off = o0 + dy * WPAD + dx
                        nc.tensor.matmul(
                            ps[g][:],
                            wh_t[g][:, tap * CHID:(tap + 1) * CHID],
                            hp[:, off:off + CHUNK_N],
                            start=(tap == 0), stop=False)
                    for tap in range(9):
                        dy, dx = divmod(tap, 3)
                        off = o0 + dy * WPAD + dx
                        nc.tensor.matmul(
                            ps[g][:],
                            wx_t[g][:, tap * CHID:(tap + 1) * CHID],
                            xp[:, off:off + CHUNK_N],
                            start=False, stop=(tap == 8))

                def pv(p):  # valid-region view of a psum chunk [128, ROWS, W]
                    return p[:].rearrange("p (r c) -> p r c", c=WPAD)[:, :, 0:W]

                def v3(t):  # [128, CC] compact -> [128, ROWS, W]
                    return t.rearrange("p (r c) -> p r c", c=W)

                ctc = outs.tile([CHID, CC], f32, tag="ct", bufs=4,
                                name=f"ct{b}_{kc}")
                nc.sync.dma_start(ctc[:], c_d[b][:, kc * CC:(kc + 1) * CC])
                csl = ctc[:]
                acts = []
                for gi in range(3):  # i, f, o with peephole + sigmoid
                    pe = work.tile([CHID, CC], f32, tag=f"pe{gi}", name=f"pe{b}_{kc}_{gi}")
                    nc.vector.tensor_mul(pe[:], peep_t[gi][:, kc * CC:(kc + 1) * CC], csl)
                    pre = work.tile([CHID, CC], f32, tag=f"pre{gi}", name=f"pre{b}_{kc}_{gi}")
                    nc.vector.tensor_add(v3(pre[:]), pv(ps[gi]), v3(pe[:]))
                    act = work.tile([CHID, CC], f32, tag=f"act{gi}", name=f"act{b}_{kc}_{gi}")
                    nc.scalar.activation(act[:], pre[:], SIG,
                                         bias=bias_t[:, gi:gi + 1])
                    acts.append(act)
                ig, fg, og = acts
                gc = work.tile([CHID, CC], f32, tag="gc")
                nc.scalar.activation(v3(gc[:]), pv(ps[3]), TANH,
                                     bias=bias_t[:, 3:4])

                t1 = work.tile([CHID, CC], f32, tag="t1")
                nc.vector.tensor_mul(t1[:], fg[:], csl)
                t2 = work.tile([CHID, CC], f32, tag="t2")
                nc.vector.tensor_mul(t2[:], ig[:], gc[:])
                cn = outs.tile([CHID, CC], f32, tag="cn")
                nc.vector.tensor_add(cn[:], t1[:], t2[:])
                hn = outs.tile([CHID, CC], f32, tag="hn")
                nc.vector.tensor_mul(hn[:], og[:], cn[:])

                sl = slice(kc * CC, (kc + 1) * CC)
                nc.sync.dma_start(og_d[b][:, sl], og[:])
                nc.sync.dma_start(cn_d[b][:, sl], cn[:])
                nc.sync.dma_start(hn_d[b][:, sl], hn[:])

    nc.compile()
    return nc


def kernel(x, h, c, Wxi, bxi, Whi, Wci, Wxf, bxf, Whf, Wcf,
           Wxo, bxo, Who, Wco, Wxc, bxc, Whc):
    global _PROG, LAST_RESULTS
    from concourse.bass_utils import run_bass_kernel_spmd

    x = np.asarray(x, dtype=np.float32)
    h = np.asarray(h, dtype=np.float32)
    c = np.asarray(c, dtype=np.float32)

    xp = _round_fp32r(_pad_flat(x))
    hp = _round_fp32r(_pad_flat(h))
    cf = np.ascontiguousarray(c.reshape(B, CHID, HW))

    # weights: [Co, Ci, 3, 3] -> [Ci, tap*Co] transposed taps, fp32r
    def wprep(w):
        w = np.asarray(w, dtype=np.float32)
        return _round_fp32r(np.ascontiguousarray(
            w.transpose(1, 2, 3, 0).reshape(w.shape[1], 9 * CHID)))

    wx = np.stack([wprep(Wxi), wprep(Wxf), wprep(Wxo), wprep(Wxc)])
    wh = np.stack([wprep(Whi), wprep(Whf), wprep(Who), wprep(Whc)])
    bias = np.stack([np.asarray(v, dtype=np.float32) for v in
                     (bxi, bxf, bxo, bxc)], axis=1)
    bias = np.ascontiguousarray(bias)  # [128, 4]
    peep = np.stack([np.asarray(v, dtype=np.float32).reshape(CHID, HW)
                     for v in (Wci, Wcf, Wco)])

    if _PROG is None:
        _PROG = _build_program()

    in_maps = []
    for i in range(N_CORES):
        sl = slice(i * PER, (i + 1) * PER)
        in_maps.append({
            "xp": np.ascontiguousarray(xp[sl]),
            "hp": np.ascontiguousarray(hp[sl]),
            "c": np.ascontiguousarray(cf[sl]),
            "wx": wx, "wh": wh, "bias": bias, "peep": peep,
        })

    res = run_bass_kernel_spmd(nc=_PROG, in_maps=in_maps,
                               core_ids=list(range(N_CORES)),
                               trace=bool(os.environ.get("KERNEL_TRACE")))
    LAST_RESULTS = res

    og = np.empty((B, CHID, HW), dtype=np.float32)
    hn = np.empty((B, CHID, HW), dtype=np.float32)
    cn = np.empty((B, CHID, HW), dtype=np.float32)
    for i in range(N_CORES):
        sl = slice(i * PER, (i + 1) * PER)
        og[sl] = res.results[i]["og"]
        hn[sl] = res.results[i]["hn"]
        cn[sl] = res.results[i]["cn"]

    shape = (B, CHID, H, W)
    return (og.reshape(shape), hn.reshape(shape), cn.reshape(shape))


# revision 4
# speedup vs baseline: 2.2779x; 2.2779x over previous
"""ConvLSTM cell forward on 8 Trainium2 NeuronCores.

Problem: B=16, Cin=64, Chid=128, H=W=64, K=3 (SAME padding).
  ig = sigmoid(conv(x,Wxi) + bxi + conv(h,Whi) + Wci*c)
  fg = sigmoid(conv(x,Wxf) + bxf + conv(h,Whf) + Wcf*c)
  c_new = fg*c + ig*tanh(conv(x,Wxc) + bxc + conv(h,Whc))
  og = sigmoid(conv(x,Wxo) + bxo + conv(h,Who) + Wco*c)
  h_new = og*c_new
  returns (og, h_new, c_new)

Strategy:
  - Data-parallel over batch: 2 images per core, weights replicated.
  - Conv as matmul over channel dim: inputs stored channel-on-partition with
    a zero-padded (H+2)x(W+2) spatial layout flattened with row stride 66.
    A 3x3 tap (dy,dx) is then a constant flat offset, so each tap is one
    contiguous matmul rhs slice accumulating into PSUM.
  - h convs: Chid=128 channels -> 9 taps of K=128 matmuls per gate.
  - x convs: Cin=64 would give K=64 matmuls, which measure ~2-3x slower per
    element on TRN2 than K=128. Instead x is stored twice on the partition
    axis: partitions 0-63 hold x_pad, partitions 64-127 hold x_pad shifted
    one padded row (+66). A K=128 matmul whose weight tile stacks the
    (dy=0,dx) tap on top of the (dy=1,dx) tap then computes both taps at
    once; the dy=2 taps use weights zero-padded to K=128. 6 x-matmuls per
    gate, all K=128.
  - Output computed in the padded-stride layout, 4 rows (N=264) per PSUM
    bank; the 2 garbage columns per row are skipped by strided views in the
    elementwise stage (DVE peephole/gate math, ScalarE sigmoid/tanh with
    per-channel bias).
  - Matmuls run in float32r (fp32 with 11-bit mantissa, ~4x fp32 speed);
    inputs/weights are pre-rounded to fp32r on the host. Elementwise math
    and PSUM accumulation stay fp32.
"""

import os
import numpy as np

B, CIN, CHID, H, W, K = 16, 64, 128, 64, 64, 3
N_CORES = 8
PER = B // N_CORES          # images per core
WPAD = W + 2                # padded row stride
FLAT = (H + 2) * WPAD + 4   # padded flat length (+4 tail pad for tap overread)
ROWS = 4                    # output rows per PSUM chunk
NCHUNK = H // ROWS
CHUNK_N = ROWS * WPAD       # 264 (even, >=256 for full-rate fp32r)
CC = ROWS * W               # 256 valid elements per chunk
HW = H * W

_PROG = None
LAST_RESULTS = None


def _round_fp32r(a):
    """Round fp32 array to fp32r (11 mantissa bits, round-half-up)."""
    b = np.ascontiguousarray(a, dtype=np.float32).view(np.uint32).astype(np.uint64)
    r = ((b + 0x800) & ~np.uint64(0xFFF)).astype(np.uint32)
    return r.view(np.float32)


def _pad_flat(a):
    """[N, C, H, W] fp32 -> [N, C, FLAT] zero-padded 66-stride layout."""
    n, c = a.shape[0], a.shape[1]
    out = np.zeros((n, c, FLAT), dtype=np.float32)
    p = out[:, :, : (H + 2) * WPAD].reshape(n, c, H + 2, WPAD)
    p[:, :, 1 : H + 1, 1 : W + 1] = a
    return out


def _build_program():
    import concourse.bacc as bacc
    import concourse.tile as tile
    import concourse.mybir as mybir
    from contextlib import ExitStack

    f32 = mybir.dt.float32
    f32r = mybir.dt.float32r

    nc = bacc.Bacc("TRN2", target_bir_lowering=False, debug=False,
                   num_devices=N_CORES)

    xp_d = nc.dram_tensor("xp", [PER, 2 * CIN, FLAT], f32r, kind="ExternalInput").ap()
    hp_d = nc.dram_tensor("hp", [PER, CHID, FLAT], f32r, kind="ExternalInput").ap()
    c_d = nc.dram_tensor("c", [PER, CHID, HW], f32, kind="ExternalInput").ap()
    # x weights: 6 K=128 tap-blocks per gate (3 stacked pairs + 3 zero-padded)
    wx_d = nc.dram_tensor("wx", [4, CHID, 6 * CHID], f32r, kind="ExternalInput").ap()
    wh_d = nc.dram_tensor("wh", [4, CHID, 9 * CHID], f32r, kind="ExternalInput").ap()
    bias_d = nc.dram_tensor("bias", [CHID, 4], f32, kind="ExternalInput").ap()
    peep_d = nc.dram_tensor("peep", [3, CHID, HW], f32, kind="ExternalInput").ap()
    og_d = nc.dram_tensor("og", [PER, CHID, HW], f32, kind="ExternalOutput").ap()
    hn_d = nc.dram_tensor("hn", [PER, CHID, HW], f32, kind="ExternalOutput").ap()
    cn_d = nc.dram_tensor("cn", [PER, CHID, HW], f32, kind="ExternalOutput").ap()

    SIG = mybir.ActivationFunctionType.Sigmoid
    TANH = mybir.ActivationFunctionType.Tanh

    # x-matmul rhs offsets within a chunk: pairs read (dy=0,dx) [the shifted
    # copy supplies dy=1], singles read (dy=2,dx)
    X_OFFS = [0, 1, 2, 2 * WPAD, 2 * WPAD + 1, 2 * WPAD + 2]

    with tile.TileContext(nc) as tc, ExitStack() as ctx:
        const = ctx.enter_context(tc.tile_pool(name="const", bufs=1))
        imgs = ctx.enter_context(tc.tile_pool(name="imgs", bufs=2))
        work = ctx.enter_context(tc.tile_pool(name="work", bufs=2))
        outs = ctx.enter_context(tc.tile_pool(name="outs", bufs=2))
        psum = ctx.enter_context(tc.tile_pool(name="psum", bufs=8, space="PSUM"))

        # image 0 DMAs first so matmuls can start ASAP
        xp0 = imgs.tile([2 * CIN, FLAT], f32r, tag="xp", name="xp0")
        hp0 = imgs.tile([CHID, FLAT], f32r, tag="hp", name="hp0")
        nc.sync.dma_start(hp0[:], hp_d[0])
        nc.sync.dma_start(xp0[:], xp_d[0])

        wx_t = [const.tile([CHID, 6 * CHID], f32r, tag=f"wx{g}", name=f"wx{g}")
                for g in range(4)]
        wh_t = [const.tile([CHID, 9 * CHID], f32r, tag=f"wh{g}", name=f"wh{g}")
                for g in range(4)]
        for g in range(4):
            nc.sync.dma_start(wh_t[g][:], wh_d[g])
            nc.sync.dma_start(wx_t[g][:], wx_d[g])
        bias_t = const.tile([CHID, 4], f32)
        nc.sync.dma_start(bias_t[:], bias_d)
        peep_t = [const.tile([CHID, HW], f32, tag=f"peep{j}", name=f"peep{j}")
                  for j in range(3)]
        for j in range(3):
            nc.sync.dma_start(peep_t[j][:], peep_d[j])

        for b in range(PER):
            if b == 0:
                xp, hp = xp0, hp0
            else:
                xp = imgs.tile([2 * CIN, FLAT], f32r, tag="xp", name=f"xp{b}")
                nc.sync.dma_start(xp[:], xp_d[b])
                hp = imgs.tile([CHID, FLAT], f32r, tag="hp", name=f"hp{b}")
                nc.sync.dma_start(hp[:], hp_d[b])

            for kc in range(NCHUNK):
                o0 = kc * CHUNK_N
                # gate order: 0=i, 1=f, 2=o, 3=candidate
                ps = [psum.tile([CHID, CHUNK_N], f32, tag="ps",
                                name=f"ps{b}_{kc}_{_g}") for _g in range(4)]
                for g in range(4):
                    for tap in range(9):
                        dy, dx = divmod(tap, 3)
                        off = o0 + dy * WPAD + dx
                        nc.tensor.matmul(
                            ps[g][:],
                            wh_t[g][:, tap * CHID:(tap + 1) * CHID],
                            hp[:, off:off + CHUNK_N],
                            start=(tap == 0), stop=False)
                    for j, xo in enumerate(X_OFFS):
                        off = o0 + xo
                        nc.tensor.matmul(
                            ps[g][:],
                            wx_t[g][:, j * CHID:(j + 1) * CHID],
                            xp[:, off:off + CHUNK_N],
                            start=False, stop=(j == 5))

                def pv(p):  # valid-region view of a psum chunk [128, ROWS, W]
                    return p[:].rearrange("p (r c) -> p r c", c=WPAD)[:, :, 0:W]

                def v3(t):  # [128, CC] compact -> [128, ROWS, W]
                    return t.rearrange("p (r c) -> p r c", c=W)

                ctc = outs.tile([CHID, CC], f32, tag="ct", bufs=4,
                                name=f"ct{b}_{kc}")
                nc.sync.dma_start(ctc[:], c_d[b][:, kc * CC:(kc + 1) * CC])
                csl = ctc[:]
                acts = []
                for gi in range(3):  # i, f, o with peephole + sigmoid
                    pe = work.tile([CHID, CC], f32, tag=f"pe{gi}",
                                   name=f"pe{b}_{kc}_{gi}")
                    nc.vector.tensor_mul(pe[:],
                                         peep_t[gi][:, kc * CC:(kc + 1) * CC], csl)
                    pre = work.tile([CHID, CC], f32, tag=f"pre{gi}",
                                    name=f"pre{b}_{kc}_{gi}")
                    nc.vector.tensor_add(v3(pre[:]), pv(ps[gi]), v3(pe[:]))
                    act = work.tile([CHID, CC], f32, tag=f"act{gi}",
                                    name=f"act{b}_{kc}_{gi}")
                    nc.scalar.activation(act[:], pre[:], SIG,
                                         bias=bias_t[:, gi:gi + 1])
                    acts.append(act)
                ig, fg, og = acts
                gc = work.tile([CHID, CC], f32, tag="gc")
                nc.scalar.activation(v3(gc[:]), pv(ps[3]), TANH,
                                     bias=bias_t[:, 3:4])

                t1 = work.tile([CHID, CC], f32, tag="t1")
                nc.vector.tensor_mul(t1[:], fg[:], csl)
                t2 = work.tile([CHID, CC], f32, tag="t2")
                nc.vector.tensor_mul(t2[:], ig[:], gc[:])
                cn = outs.tile([CHID, CC], f32, tag="cn")
                nc.vector.tensor_add(cn[:], t1[:], t2[:])
                hn = outs.tile([CHID, CC], f32, tag="hn")
                nc.vector.tensor_mul(hn[:], og[:], cn[:])

                sl = slice(kc * CC, (kc + 1) * CC)
                nc.sync.dma_start(og_d[b][:, sl], og[:])
                nc.sync.dma_start(cn_d[b][:, sl], cn[:])
                nc.sync.dma_start(hn_d[b][:, sl], hn[:])

    nc.compile()
    return nc


def kernel(x, h, c, Wxi, bxi, Whi, Wci, Wxf, bxf, Whf, Wcf,
           Wxo, bxo, Who, Wco, Wxc, bxc, Whc):
    global _PROG, LAST_RESULTS
    from concourse.bass_utils import run_bass_kernel_spmd

    x = np.asarray(x, dtype=np.float32)
    h = np.asarray(h, dtype=np.float32)
    c = np.asarray(c, dtype=np.float32)

    # x: padded layout duplicated on the channel axis, second copy shifted
    # one padded row so a K=128 matmul covers (dy=0, dy=1) tap pairs
    xpad = _pad_flat(x)
    xp = np.zeros((B, 2 * CIN, FLAT), dtype=np.float32)
    xp[:, :CIN] = xpad
    xp[:, CIN:, : FLAT - WPAD] = xpad[:, :, WPAD:]
    xp = _round_fp32r(xp)
    hp = _round_fp32r(_pad_flat(h))
    cf = np.ascontiguousarray(c.reshape(B, CHID, HW))

    def wx_prep(w):
        # [Co=128, Ci=64, 3, 3] -> [128, 6*128]: blocks 0-2 stack the
        # (dy=0,dx) tap over (dy=1,dx); blocks 3-5 hold (dy=2,dx) over zeros
        w = np.asarray(w, dtype=np.float32)
        out = np.zeros((CHID, 6 * CHID), dtype=np.float32)
        for dx in range(3):
            out[:CIN, dx * CHID:(dx + 1) * CHID] = w[:, :, 0, dx].T
            out[CIN:, dx * CHID:(dx + 1) * CHID] = w[:, :, 1, dx].T
            out[:CIN, (3 + dx) * CHID:(4 + dx) * CHID] = w[:, :, 2, dx].T
        return _round_fp32r(out)

    def wh_prep(w):
        w = np.asarray(w, dtype=np.float32)
        return _round_fp32r(np.ascontiguousarray(
            w.transpose(1, 2, 3, 0).reshape(CHID, 9 * CHID)))

    wx = np.stack([wx_prep(Wxi), wx_prep(Wxf), wx_prep(Wxo), wx_prep(Wxc)])
    wh = np.stack([wh_prep(Whi), wh_prep(Whf), wh_prep(Who), wh_prep(Whc)])
    bias = np.ascontiguousarray(np.stack(
        [np.asarray(v, dtype=np.float32) for v in (bxi, bxf, bxo, bxc)], axis=1))
    peep = np.stack([np.asarray(v, dtype=np.float32).reshape(CHID, HW)
                     for v in (Wci, Wcf, Wco)])

    if _PROG is None:
        _PROG = _build_program()

    in_maps = []
    for i in range(N_CORES):
        sl = slice(i * PER, (i + 1) * PER)
        in_maps.append({
            "xp": np.ascontiguousarray(xp[sl]),
            "hp": np.ascontiguousarray(hp[sl]),
            "c": np.ascontiguousarray(cf[sl]),
            "wx": wx, "wh": wh, "bias": bias, "peep": peep,
        })

    res = run_bass_kernel_spmd(nc=_PROG, in_maps=in_maps,
                               core_ids=list(range(N_CORES)),
                               trace=bool(os.environ.get("KERNEL_TRACE")))
    LAST_RESULTS = res

    og = np.empty((B, CHID, HW), dtype=np.float32)
    hn = np.empty((B, CHID, HW), dtype=np.float32)
    cn = np.empty((B, CHID, HW), dtype=np.float32)
    for i in range(N_CORES):
        sl = slice(i * PER, (i + 1) * PER)
        og[sl] = res.results[i]["og"]
        hn[sl] = res.results[i]["hn"]
        cn[sl] = res.results[i]["cn"]

    shape = (B, CHID, H, W)
    return (og.reshape(shape), hn.reshape(shape), cn.reshape(shape))
